# revision 29
# baseline (speedup 1.0000x reference)
"""Multi-head self-attention TRN2 kernel (8 NeuronCores, head-parallel).

Problem: x[L=4096, N=1, E=1024], w_qkv[3E, E], w_out[E, E], H=16 heads, DH=64.
Sharding: 2 heads per core (128 q/k/v dims). Each core computes its heads'
attention and a partial out-projection; host sums the 8 partials.

Per-core algorithm (all matmuls bf16, fp32 PSUM accumulation):
  qT[d,l] = wqT.T @ xT   (scale folded into wq on host)
  kT[d,l] = wkT.T @ xT
  V[l,d]  = xT.T @ wvT   (stored as V_aug = [V_A|1|V_B|1] for the PV matmul)
  For each query chunk (512 cols) and each key tile jt (128 rows):
    ST[j, i]  = kT[:,jt].T @ qT[:,chunk]   (both heads row-tiled on the PE)
    PT        = exp(ST)                    (no max subtraction: |S| <= ~5)
    O_h[65,i] += [V_h|1].T @ PT_h          (row 64 = softmax denominator)
  OTn[d,i] = O_h[0:64] * broadcast(1/denom) ; out = OTn.T @ woT per l-tile.
"""

import sys
import os
import numpy as np

try:
    import concourse.bass as bass  # noqa: F401
except ImportError:
    sys.path.insert(0, "/opt/trn_rl_repo")

import ml_dtypes
import concourse.bass as bass
import concourse.mybir as mybir
import concourse.tile as tile
from concourse import bacc
from concourse.bass_utils import run_bass_kernel_spmd

BF16 = mybir.dt.bfloat16
F32 = mybir.dt.float32
AF = mybir.ActivationFunctionType

L, N, E, H = 4096, 1, 1024, 16
DH = E // H            # 64
P = 128                # partitions / dims per core (2 heads)
SCALE = DH ** -0.5
NCORES = 8
ET = E // P            # 8 contraction tiles for the projections


def build(nc, L=L):
    LT = L // P            # key tiles
    CH = L // 512          # query chunks of 512
    CW = 512               # chunk width

    xT_d = nc.declare_dram_parameter("xT", [E, L], BF16, isOutput=False)
    wqT_d = nc.declare_dram_parameter("wqT", [E, P], BF16, isOutput=False)
    wkT_d = nc.declare_dram_parameter("wkT", [E, P], BF16, isOutput=False)
    wvT_d = nc.declare_dram_parameter("wvT", [E, P], BF16, isOutput=False)
    woT_d = nc.declare_dram_parameter("woT", [P, E], BF16, isOutput=False)
    out_d = nc.declare_dram_parameter("out", [L, E], BF16, isOutput=True)

    xT_t = xT_d.ap().rearrange("(t p) l -> p t l", p=P)
    wq_t = wqT_d.ap().rearrange("(t p) d -> p t d", p=P)
    wk_t = wkT_d.ap().rearrange("(t p) d -> p t d", p=P)
    wv_t = wvT_d.ap().rearrange("(t p) d -> p t d", p=P)
    out_t = out_d.ap().rearrange("(t p) f -> p t f", p=P)

    with tile.TileContext(nc) as tc:
        with (
            tc.tile_pool(name="persist", bufs=1) as sbp,
            tc.tile_pool(name="pt", bufs=18) as sb_pt,
            tc.tile_pool(name="ob", bufs=1) as sb_ob,
            tc.tile_pool(name="misc", bufs=1) as sb_misc,
            tc.tile_pool(name="psbig", bufs=2, space="PSUM") as ps_big,
            tc.tile_pool(name="psone", bufs=2, space="PSUM") as ps_one,
            tc.tile_pool(name="pso", bufs=2, space="PSUM") as ps_o,
        ):
            # ---- persistent SBUF tiles + input DMAs ----
            # Each dma_start costs ~650 ns of serial issue time on the sync
            # engine, so batch aggressively with 3D access patterns: one DMA
            # per weight tensor, one per xT column block.
            wq_sb = sbp.tile([P, ET, P], BF16, tag="wq")
            wk_sb = sbp.tile([P, ET, P], BF16, tag="wk")
            wv_sb = sbp.tile([P, ET, P], BF16, tag="wv")
            nc.sync.dma_start(out=wk_sb, in_=wk_t)
            nc.sync.dma_start(out=wq_sb, in_=wq_t)

            # xT loaded in column blocks, one tile per block so each
            # projection chunk depends only on its own 1 MB of DMA traffic
            # instead of the full 8.4 MB transfer.  Block 0 is queued right
            # after wk/wq so the first projection can start ASAP.
            xT_sb = []
            for lc in range(CH):
                t = sbp.tile([P, ET, CW], BF16, tag=f"xtb{lc}")
                nc.sync.dma_start(out=t, in_=xT_t[:, :, lc * CW:(lc + 1) * CW])
                xT_sb.append(t)
                if lc == 0:
                    nc.sync.dma_start(out=wv_sb, in_=wv_t)
            wo_sb = sbp.tile([P, E], BF16, tag="wo")
            nc.sync.dma_start(out=wo_sb, in_=woT_d.ap())

            # Dummy matmuls on a zeroed scratch tile during the initial DMA
            # wait: keeps the PE's HAM activity monitor busy so the clock is
            # already at 2.4 GHz when the first real projection lands.
            warm = sbp.tile([P, CW], BF16, tag="warm")
            nc.vector.memset(warm, 0.0)
            for _ in range(30):
                wp = ps_one.tile([P, CW], F32, tag="p1")
                nc.tensor.matmul(wp, lhsT=warm[:, 0:P], rhs=warm,
                                 start=True, stop=True)

            V_aug = []
            for lt in range(LT):
                t = sbp.tile([P, 2 * DH + 2], BF16, tag=f"va{lt}")
                nc.vector.memset(t[:, DH:DH + 1], 1.0)
                nc.vector.memset(t[:, 2 * DH + 1:2 * DH + 2], 1.0)
                V_aug.append(t)

            qT = sbp.tile([P, L], BF16, tag="qT")
            kT = sbp.tile([P, L], BF16, tag="kT")
            OTn = sbp.tile([P, L], BF16, tag="otn")

            # ---- projections (mostly emitted as per-slot fillers inside the
            # attention stream so the ScalarE exp pipeline starts early) ----
            def proj_chunk(dst, w, lc):
                ps = ps_one.tile([P, CW], F32, tag="p1")
                for e in range(ET):
                    nc.tensor.matmul(
                        ps, lhsT=w[:, e, :], rhs=xT_sb[lc][:, e, :],
                        start=(e == 0), stop=(e == ET - 1))
                nc.vector.tensor_copy(out=dst[:, lc * CW:(lc + 1) * CW], in_=ps)

            def v_tile(lt):
                lc, off = lt // (CW // P), (lt % (CW // P)) * P
                def emit():
                    ps = ps_one.tile([P, P], F32, tag="p1")
                    for e in range(ET):
                        nc.tensor.matmul(
                            ps, lhsT=xT_sb[lc][:, e, off:off + P],
                            rhs=wv_sb[:, e, :], start=(e == 0), stop=(e == ET - 1))
                    nc.vector.tensor_copy(out=V_aug[lt][:, 0:DH], in_=ps[:, 0:DH])
                    nc.vector.tensor_copy(out=V_aug[lt][:, DH + 1:2 * DH + 1],
                                          in_=ps[:, DH:2 * DH])
                return emit

            fillers = []
            for lc in range(1, CH):
                fillers.append(lambda lc=lc: proj_chunk(kT, wk_sb, lc))
            for lt in range(LT):
                fillers.append(v_tile(lt))

            proj_chunk(kT, wk_sb, 0)
            proj_chunk(qT, wq_sb, 0)

            # ---- phase 2: attention ----
            def emit_scores(c, jt):
                st = ps_big.tile([P, 2 * CW], F32, tag="st")
                nc.tensor.matmul(
                    st[:, 0:CW], lhsT=kT[0:DH, jt * P:(jt + 1) * P],
                    rhs=qT[0:DH, c * CW:(c + 1) * CW], start=True, stop=True)
                nc.tensor.matmul(
                    st[:, CW:2 * CW], lhsT=kT[DH:P, jt * P:(jt + 1) * P],
                    rhs=qT[DH:P, c * CW:(c + 1) * CW], start=True, stop=True)
                return st

            def outproj_unit(c, lt, fc, stage):
                # out[l, f] for l-tile lt of chunk c, f columns [fc*512, +512)
                def emit():
                    glt = c * (CW // P) + lt
                    po = ps_one.tile([P, CW], F32, tag="p1")
                    nc.tensor.matmul(
                        po, lhsT=OTn[:, glt * P:(glt + 1) * P],
                        rhs=wo_sb[:, fc * CW:(fc + 1) * CW], start=True, stop=True)
                    nc.vector.tensor_copy(
                        out=stage[:, lt, fc * CW:(fc + 1) * CW], in_=po)
                return emit

            def outproj_flush(c, stage):
                def emit():
                    nt = CW // P
                    nc.sync.dma_start(
                        out=out_t[:, c * nt:(c + 1) * nt, :], in_=stage)
                return emit

            def emit_epilogue(c, o_a, o_b):
                # copy O to SBUF first (frees the PSUM banks so the next
                # chunk's PV can start; keeps the PE dense so the HAM clock
                # stays at 2.4 GHz), then normalize off the critical path.
                # Two per-head chains, interleaved so DVE / DMA / GpSimd steps
                # of head A overlap head B's.  Denominator rows live at
                # partition 64; custom-DVE ops can't shift partitions, so DMA
                # them to partition 0 first.
                oa_sb = sb_misc.tile([DH + 1, CW], F32, tag="oasb")
                ob_sb = sb_misc.tile([DH + 1, CW], F32, tag="obsb")
                dna = sb_misc.tile([1, CW], F32, tag="dna")
                dnb = sb_misc.tile([1, CW], F32, tag="dnb")
                raa = sb_misc.tile([1, CW], F32, tag="raa")
                rab = sb_misc.tile([1, CW], F32, tag="rab")
                bca = sb_misc.tile([DH, CW], F32, tag="bca")
                bcb = sb_misc.tile([DH, CW], F32, tag="bcb")
                nc.vector.tensor_copy(out=oa_sb, in_=o_a)
                nc.sync.dma_start(out=dna, in_=oa_sb[DH:DH + 1, :])
                nc.vector.tensor_copy(out=ob_sb, in_=o_b)
                nc.vector.reciprocal_approx_fast(out=raa, in_=dna)
                nc.sync.dma_start(out=dnb, in_=ob_sb[DH:DH + 1, :])
                nc.gpsimd.partition_broadcast(bca, raa)
                nc.vector.reciprocal_approx_fast(out=rab, in_=dnb)
                nc.vector.tensor_mul(
                    out=OTn[0:DH, c * CW:(c + 1) * CW],
                    in0=oa_sb[0:DH, :], in1=bca)
                nc.gpsimd.partition_broadcast(bcb, rab)
                otb = sb_misc.tile([DH, CW], BF16, tag="otb")
                nc.vector.tensor_mul(out=otb, in0=ob_sb[0:DH, :], in1=bcb)
                # partition shift 0:64 -> 64:128 via SBUF->SBUF DMA
                nc.sync.dma_start(out=OTn[DH:P, c * CW:(c + 1) * CW], in_=otb)
                stage = sb_ob.tile([P, CW // P, E], BF16, tag="ob")
                for lt in range(CW // P):
                    for fc in range(E // CW):
                        deferred.append(outproj_unit(c, lt, fc, stage))
                deferred.append(outproj_flush(c, stage))

            def mk_pv(c, jt, pt, od):
                # PV matmuls for chunk c, key tile jt. O tiles allocated on
                # first use so their PSUM banks are claimed only when the
                # deferred stream actually starts.
                def emit():
                    if "t" not in od:
                        a = ps_o.tile([DH + 1, CW], F32, tag="o")
                        b = ps_o.tile([DH + 1, CW], F32, tag="o")
                        od["t"] = (a, b)
                    o_a, o_b = od["t"]
                    nc.tensor.matmul(
                        o_a, lhsT=V_aug[jt][:, 0:DH + 1], rhs=pt[:, 0:CW],
                        start=(jt == 0), stop=(jt == LT - 1))
                    nc.tensor.matmul(
                        o_b, lhsT=V_aug[jt][:, DH + 1:2 * DH + 2],
                        rhs=pt[:, CW:2 * CW],
                        start=(jt == 0), stop=(jt == LT - 1))
                return emit

            # Software pipeline: PV(c, jt) executes D slots after exp(c, jt),
            # so the ScalarE exp stream never waits on V/K production (which
            # rides along as fillers in the early slots).  The last chunk
            # drains the queue gradually so PE never bursts while ACT idles.
            D = 14 if CH > 1 else 0
            deferred = []
            pending = []          # (c, jt, pv-closure)
            ods = {c: {} for c in range(CH)}

            def pop_pv():
                pc, pjt, f = pending.pop(0)
                f()
                if pjt == LT - 1:
                    emit_epilogue(pc, *ods[pc]["t"])

            st_cur = emit_scores(0, 0)
            for c in range(CH):
                last = (c == CH - 1)
                for jt in range(LT):
                    pt = sb_pt.tile([P, 2 * CW], BF16, tag="pt")
                    nc.scalar.activation(out=pt, in_=st_cur, func=AF.Exp)
                    if jt < LT - 1:
                        st_next = emit_scores(c, jt + 1)
                    elif not last:
                        st_next = emit_scores(c + 1, 0)
                    if fillers:
                        fillers.pop(0)()
                    pending.append((c, jt, mk_pv(c, jt, pt, ods[c])))
                    limit = D if not last else max(0, D - 2 * (jt + 1))
                    while len(pending) > limit:
                        pop_pv()
                    if deferred and jt % 3 == 2:
                        deferred.pop(0)()
                    if jt == 10 and c + 1 < CH:
                        proj_chunk(qT, wq_sb, c + 1)
                    if jt < LT - 1 or not last:
                        st_cur = st_next
            while pending:
                pop_pv()
            for f in deferred:
                f()
    nc.finalize()
    return nc


_built = {}


def _get_nc(l=L):
    if l not in _built:
        nc = bacc.Bacc()
        _built[l] = build(nc, l)
    return _built[l]


def _prep_inputs(x, w_qkv, w_out, l=L):
    x2 = np.asarray(x, dtype=np.float32).reshape(l, E)
    xT = np.ascontiguousarray(x2.T).astype(ml_dtypes.bfloat16)
    wq, wk, wv = w_qkv[0:E], w_qkv[E:2 * E], w_qkv[2 * E:3 * E]
    in_maps = []
    for c in range(NCORES):
        d0 = c * P
        in_maps.append({
            "xT": xT,
            "wqT": np.ascontiguousarray(
                (wq[d0:d0 + P] * SCALE).T).astype(ml_dtypes.bfloat16),
            "wkT": np.ascontiguousarray(wk[d0:d0 + P].T).astype(ml_dtypes.bfloat16),
            "wvT": np.ascontiguousarray(wv[d0:d0 + P].T).astype(ml_dtypes.bfloat16),
            "woT": np.ascontiguousarray(
                w_out[:, d0:d0 + P].T).astype(ml_dtypes.bfloat16),
        })
    return in_maps


def _run(x, w_qkv, w_out, l=L, **kw):
    nc = _get_nc(l)
    in_maps = _prep_inputs(x, w_qkv, w_out, l)
    res = run_bass_kernel_spmd(nc, in_maps, core_ids=list(range(NCORES)), **kw)
    acc = np.zeros((l, E), dtype=np.float32)
    for r in res.results:
        acc += r["out"].astype(np.float32)
    return acc.reshape(l, N, E), res


def kernel(x, w_qkv, w_out):
    out, _ = _run(x, w_qkv, w_out)
    return out
